# revision 11
# baseline (speedup 1.0000x reference)
"""T5 transformer block (RMSNorm->MHA+bias->residual->RMSNorm->FFN->residual)
on 8 Trainium2 NeuronCores, data-parallel over batch (B=8, one element/core).

kernel(**inputs) takes FULL unsharded inputs, returns FULL [8,1024,512] output.
"""

import os
import sys
from contextlib import ExitStack

import numpy as np

if not any(os.path.isdir(os.path.join(p, "concourse")) for p in sys.path if p):
    sys.path.insert(0, "/opt/trn_rl_repo")

import concourse.bass as bass
import concourse.mybir as mybir
import concourse.tile as tile
from concourse import bacc
from concourse.bass_utils import run_bass_kernel_spmd
from concourse.masks import make_identity

FP32 = mybir.dt.float32
BF16 = mybir.dt.bfloat16
AF = mybir.ActivationFunctionType

B, S, D, H, HD, DFF = 8, 1024, 512, 8, 64, 2048
EPS = 1e-6
P = 128
T = S // P    # 8 sequence tiles
DC = D // P   # 4 d-chunks
FC = DFF // P # 16 ff-chunks
NH = 512      # matmul moving free dim


def _load_cast_weight(nc, pool, dram, rows, cols, name):
    """DRAM [rows, cols] f32 -> SBUF [128, rows//128, cols] bf16 (cast in DMA)."""
    t = pool.tile([P, rows // P, cols], BF16, tag="wraw")
    src = dram[:, :].rearrange("(j p) d -> p j d", p=P)
    nc.gpsimd.dma_start(out=t[:], in_=src)
    return t


def _transpose_to(nc, psum_pool, out_tile, in_tile, ident, evac="vector"):
    """in_tile [128, J, cols] bf16 -> out_tile[:, c, :] = transpose per 128-block.

    in (j, 128c:128c+128) block -> out (c, 128j:128j+128).
    """
    J = in_tile.shape[1]
    C = in_tile.shape[2] // P
    for c in range(C):
        pt = psum_pool.tile([P, J * P], BF16, tag="ptrans")
        for j in range(J):
            nc.tensor.transpose(
                pt[:, j * P:(j + 1) * P],
                in_tile[:, j, c * P:(c + 1) * P],
                ident[:],
            )
        if evac == "vector":
            nc.vector.tensor_copy(out_tile[:, c, :], pt[:])
        else:
            nc.scalar.copy(out_tile[:, c, :], pt[:])


def _rmsnorm_transposed(nc, tc, pools, x_sb, w_sb, out_tT, xn_tile, ident,
                        eps_sb):
    """x_sb [128, T, 512] f32 -> out_tT [128, DC, 1024] bf16 = (w * x/rms(x))^T."""
    scr_pool, stat_pool, pt_pool = pools
    ss = stat_pool.tile([P, T], FP32, tag="ss")
    sst = stat_pool.tile([P, T], FP32, tag="sst")
    rinv = stat_pool.tile([P, T], FP32, tag="rinv")
    for t in range(T):
        scr = scr_pool.tile([P, D], FP32, tag="sqscr")
        nc.scalar.activation(scr[:], x_sb[:, t, :], AF.Square,
                             accum_out=ss[:, t:t + 1])
    nc.scalar.activation(sst[:], ss[:], AF.Sqrt, bias=eps_sb[:], scale=1.0 / D)
    nc.vector.reciprocal(rinv[:], sst[:])
    for t in range(T):
        nc.vector.tensor_scalar_mul(xn_tile[:, t, :], x_sb[:, t, :],
                                    rinv[:, t:t + 1])
    # transpose xn -> out_tT, folding per-feature weight w (per-partition there)
    for c in range(DC):
        pt = pt_pool.tile([P, S], BF16, tag="ptrans")
        for t in range(T):
            nc.tensor.transpose(pt[:, t * P:(t + 1) * P],
                                xn_tile[:, t, c * P:(c + 1) * P], ident[:])
        nc.vector.tensor_scalar_mul(out_tT[:, c, :], pt[:], w_sb[:, c:c + 1])


def build_bass():
    nc = bacc.Bacc("TRN2", target_bir_lowering=False, debug=False,
                   num_devices=8)
    dr = {}
    dr["wk"] = nc.dram_tensor("primals_1", [D, D], FP32, kind="ExternalInput")
    dr["wo"] = nc.dram_tensor("primals_2", [D, D], FP32, kind="ExternalInput")
    dr["wq"] = nc.dram_tensor("primals_3", [D, D], FP32, kind="ExternalInput")
    dr["wv"] = nc.dram_tensor("primals_4", [D, D], FP32, kind="ExternalInput")
    dr["w1"] = nc.dram_tensor("primals_5", [D], FP32, kind="ExternalInput")
    dr["wi"] = nc.dram_tensor("primals_6", [DFF, D], FP32, kind="ExternalInput")
    dr["wf"] = nc.dram_tensor("primals_7", [D, DFF], FP32, kind="ExternalInput")
    dr["w2"] = nc.dram_tensor("primals_8", [D], FP32, kind="ExternalInput")
    dr["x"] = nc.dram_tensor("primals_9", [S, D], FP32, kind="ExternalInput")
    dr["bias"] = nc.dram_tensor("primals_10", [H, S, S], FP32,
                                kind="ExternalInput")
    out_dram = nc.dram_tensor("out", [S, D], FP32, kind="ExternalOutput")

    with tile.TileContext(nc) as tc:
        with ExitStack() as ctx:
            build_kernel(ctx, tc, dr, out_dram)
    nc.compile()
    return nc


def build_kernel(ctx, tc, dr, out_dram):
    nc = tc.nc

    const_pool = ctx.enter_context(tc.tile_pool(name="const", bufs=1))
    main_pool = ctx.enter_context(tc.tile_pool(name="main", bufs=1))
    stat_pool = ctx.enter_context(tc.tile_pool(name="stat", bufs=1))
    tiny_pool = ctx.enter_context(tc.tile_pool(name="tiny", bufs=8))

    ident = const_pool.tile([P, P], BF16)
    make_identity(nc, ident[:])
    eps_sb = const_pool.tile([P, 1], FP32)
    nc.gpsimd.memset(eps_sb[:], EPS)
    w1_sb = const_pool.tile([P, DC], FP32)
    nc.sync.dma_start(out=w1_sb[:], in_=dr["w1"][:].rearrange("(c p) -> p c", p=P))
    w2_sb = const_pool.tile([P, DC], FP32)
    nc.sync.dma_start(out=w2_sb[:], in_=dr["w2"][:].rearrange("(c p) -> p c", p=P))

    x_sb = main_pool.tile([P, T, D], FP32)
    nc.sync.dma_start(out=x_sb[:], in_=dr["x"][:, :].rearrange("(t p) d -> p t d", p=P))
    y_sb = main_pool.tile([P, T, D], FP32)

    with tc.tile_pool(name="woT", bufs=1) as woT_pool:
        WoT = woT_pool.tile([P, DC, D], BF16)
        with tc.tile_pool(name="qkv", bufs=1) as qkv_pool:
            hT = qkv_pool.tile([P, DC, S], BF16)
            QT = qkv_pool.tile([P, DC, S], BF16)
            KT = qkv_pool.tile([P, DC, S], BF16)
            V_aug = qkv_pool.tile([P, T, H * (HD + 1)], BF16)
            nc.gpsimd.memset(V_aug[:], 1.0)

            # ---- stage A: attention weights: load (cast bf16) + transpose
            with tc.tile_pool(name="wqkvT", bufs=1) as wqkvT_pool, \
                 tc.tile_pool(name="wstage", bufs=2) as wstage_pool, \
                 tc.tile_pool(name="pw", bufs=2, space="PSUM") as pw_pool:
                WqT = wqkvT_pool.tile([P, DC, D], BF16)
                WkT = wqkvT_pool.tile([P, DC, D], BF16)
                WvT = wqkvT_pool.tile([P, DC, D], BF16)
                for wdram, wT in ((dr["wq"], WqT), (dr["wk"], WkT),
                                  (dr["wv"], WvT), (dr["wo"], WoT)):
                    raw = _load_cast_weight(nc, wstage_pool, wdram, D, D, "w")
                    _transpose_to(nc, pw_pool, wT, raw, ident)

                # ---- stage B: rmsnorm1 + transpose -> hT
                with tc.tile_pool(name="pscr", bufs=2, space="PSUM") as scr_pool:
                    xn = main_pool.tile([P, T, D], BF16, tag="sd_bf16")
                    _rmsnorm_transposed(nc, tc, (scr_pool, stat_pool, pw_pool),
                                        x_sb, w1_sb, hT, xn, ident, eps_sb)

                # ---- stage C: Q^T, K^T (transposed), V (normal, augmented)
                with tc.tile_pool(name="pqkv", bufs=3, space="PSUM") as pq_pool:
                    for wT, dstT in ((WqT, QT), (WkT, KT)):
                        for j in range(DC):        # output e-chunk
                            for n in range(S // NH):
                                pq = pq_pool.tile([P, NH], FP32, tag="pq")
                                for c in range(DC):
                                    nc.tensor.matmul(
                                        pq[:],
                                        wT[:, c, j * P:(j + 1) * P],
                                        hT[:, c, n * NH:(n + 1) * NH],
                                        start=(c == 0), stop=(c == DC - 1))
                                nc.scalar.copy(dstT[:, j, n * NH:(n + 1) * NH], pq[:])
                    for t in range(T):
                        pv = pq_pool.tile([P, D], FP32, tag="pq")
                        for c in range(DC):
                            nc.tensor.matmul(pv[:], hT[:, c, t * P:(t + 1) * P],
                                             WvT[:, c, :],
                                             start=(c == 0), stop=(c == DC - 1))
                        # scatter heads into V_aug (col 64 of each head stays 1.0)
                        vdst = V_aug[:, t, :].rearrange("p (h v) -> p h v", v=HD + 1)
                        vsrc = pv[:].rearrange("p (h w) -> p h w", w=HD)
                        nc.vector.tensor_copy(vdst[:, :, 0:HD], vsrc)
            # wqkvT/wstage/psum pools closed

            # ---- stage D: attention, software-pipelined over head pairs
            ctx_sb = main_pool.tile([P, T, D], BF16, tag="sd_bf16")
            NP_ = H // 2  # 4 pairs
            with tc.tile_pool(name="sc", bufs=4) as sc_pool, \
                 tc.tile_pool(name="biasp", bufs=3) as bias_pool, \
                 tc.tile_pool(name="probsT", bufs=2) as pT_pool, \
                 tc.tile_pool(name="ps", bufs=2, space="PSUM") as ps_pool, \
                 tc.tile_pool(name="ppt", bufs=2, space="PSUM") as ppt_pool, \
                 tc.tile_pool(name="pctx", bufs=2, space="PSUM") as pctx_pool:

                sc_tiles = {}

                def trace_scores(p, t):
                    # row-packed pair: head h uses partitions 64*(h%2).. of
                    # Q^T/K^T chunk p (QT[:, p, :] holds heads 2p, 2p+1)
                    for hh in range(2):
                        h = 2 * p + hh
                        lo = 64 * hh
                        bias_t = bias_pool.tile([P, S], FP32, tag="bias")
                        nc.sync.dma_start(
                            out=bias_t[:],
                            in_=dr["bias"][h, t * P:(t + 1) * P, :])
                        psc = ps_pool.tile([P, S], FP32, tag="ps")
                        for n in range(S // NH):
                            nc.tensor.matmul(
                                psc[:, n * NH:(n + 1) * NH],
                                QT[lo:lo + HD, p, t * P:(t + 1) * P],
                                KT[lo:lo + HD, p, n * NH:(n + 1) * NH],
                                start=True, stop=True)
                        sc = sc_tiles[(p, hh)]
                        nc.vector.tensor_add(sc[:, t, :], psc[:], bias_t[:])

                def trace_transposes(p, hh, kc):
                    h = 2 * p + hh
                    sc = sc_tiles[(p, hh)]
                    ppt = ppt_pool.tile([P, S], BF16, tag="ppt")
                    for t in range(T):
                        nc.tensor.transpose(
                            ppt[:, t * P:(t + 1) * P],
                            sc[:, t, kc * P:(kc + 1) * P], ident[:])
                    probsT = sc_tiles[("pT", p, hh)]
                    nc.scalar.activation(probsT[:, kc, :], ppt[:], AF.Exp)

                def trace_ctx(p, hh, t):
                    h = 2 * p + hh
                    probsT = sc_tiles[("pT", p, hh)]
                    pc = pctx_pool.tile([P, HD + 1], FP32, tag="pctx")
                    for kc in range(T):
                        nc.tensor.matmul(
                            pc[:],
                            probsT[:, kc, t * P:(t + 1) * P],
                            V_aug[:, kc, h * (HD + 1):(h + 1) * (HD + 1)],
                            start=(kc == 0), stop=(kc == T - 1))
                    rz = tiny_pool.tile([P, 1], FP32, tag="rz")
                    nc.vector.reciprocal(rz[:], pc[:, HD:HD + 1])
                    nc.vector.tensor_scalar_mul(
                        ctx_sb[:, t, h * HD:(h + 1) * HD], pc[:, 0:HD], rz[:])

                for it in range(NP_ + 1):
                    if it < NP_:
                        for hh in range(2):
                            sc_tiles[(it, hh)] = sc_pool.tile(
                                [P, T, S], BF16, tag="sc", name=f"sc_{it}_{hh}")
                    if it > 0:
                        for hh in range(2):
                            sc_tiles[("pT", it - 1, hh)] = pT_pool.tile(
                                [P, T, S], BF16, tag="pT", name=f"pT_{it}_{hh}")
                    for t in range(T):
                        if it < NP_:
                            trace_scores(it, t)
                        if it > 0:
                            trace_transposes(it - 1, 0, t)
                            trace_transposes(it - 1, 1, t)
                    if it > 0:
                        for hh in range(2):
                            for t in range(T):
                                trace_ctx(it - 1, hh, t)

        # qkv pool closed. ---- stage E: ctx^T + O-proj + residual
        with tc.tile_pool(name="epool", bufs=1) as e_pool, \
             tc.tile_pool(name="pct", bufs=2, space="PSUM") as pct_pool, \
             tc.tile_pool(name="po", bufs=3, space="PSUM") as po_pool:
            ctxT = e_pool.tile([P, DC, S], BF16)
            _transpose_to(nc, pct_pool, ctxT, ctx_sb, ident, evac="scalar")
            for t in range(T):
                po = po_pool.tile([P, D], FP32, tag="po")
                for c in range(DC):
                    nc.tensor.matmul(po[:], ctxT[:, c, t * P:(t + 1) * P],
                                     WoT[:, c, :],
                                     start=(c == 0), stop=(c == DC - 1))
                nc.vector.tensor_add(y_sb[:, t, :], po[:], x_sb[:, t, :])
    # woT closed

    # ---- stage F: rmsnorm2 + FFN weight prep
    with tc.tile_pool(name="ffnw", bufs=1) as ffnw_pool, \
         tc.tile_pool(name="ffn", bufs=1) as ffn_pool:
        wiT = ffnw_pool.tile([P, DC, DFF], BF16)
        woffT = ffnw_pool.tile([P, FC, D], BF16)
        h2T = ffn_pool.tile([P, DC, S], BF16)
        with tc.tile_pool(name="fstage", bufs=2) as fstage_pool, \
             tc.tile_pool(name="pwf", bufs=2, space="PSUM") as pwf_pool, \
             tc.tile_pool(name="pscr2", bufs=2, space="PSUM") as scr2_pool:
            h2n = ffn_pool.tile([P, T, D], BF16)
            _rmsnorm_transposed(nc, tc, (scr2_pool, stat_pool, pwf_pool),
                                y_sb, w2_sb, h2T, h2n, ident, eps_sb)
            raw_wi = _load_cast_weight(nc, fstage_pool, dr["wi"], DFF, D, "wi")
            _transpose_to(nc, pwf_pool, wiT, raw_wi, ident)
            raw_wf = fstage_pool.tile([P, DC, DFF], BF16, tag="wraw")
            nc.gpsimd.dma_start(
                out=raw_wf[:],
                in_=dr["wf"][:, :].rearrange("(c p) f -> p c f", p=P))
            _transpose_to(nc, pwf_pool, woffT, raw_wf, ident)

        # ---- stage G: FFN
        ffT = ffn_pool.tile([P, FC, S], BF16)
        with tc.tile_pool(name="pf", bufs=3, space="PSUM") as pf_pool, \
             tc.tile_pool(name="pff", bufs=2, space="PSUM") as pff_pool, \
             tc.tile_pool(name="outp", bufs=3) as out_pool:
            for j in range(FC):
                for n in range(S // NH):
                    pf = pf_pool.tile([P, NH], FP32, tag="pf")
                    for c in range(DC):
                        nc.tensor.matmul(pf[:], wiT[:, c, j * P:(j + 1) * P],
                                         h2T[:, c, n * NH:(n + 1) * NH],
                                         start=(c == 0), stop=(c == DC - 1))
                    if j % 2 == 0:
                        nc.scalar.activation(ffT[:, j, n * NH:(n + 1) * NH],
                                             pf[:], AF.Relu)
                    else:
                        nc.vector.tensor_scalar_max(
                            ffT[:, j, n * NH:(n + 1) * NH], pf[:], 0.0)
            for t in range(T):
                pff = pff_pool.tile([P, D], FP32, tag="pff")
                for j in range(FC):
                    nc.tensor.matmul(pff[:], ffT[:, j, t * P:(t + 1) * P],
                                     woffT[:, j, :],
                                     start=(j == 0), stop=(j == FC - 1))
                out_t = out_pool.tile([P, D], FP32, tag="out")
                nc.vector.tensor_add(out_t[:], pff[:], y_sb[:, t, :])
                nc.sync.dma_start(out=out_dram[t * P:(t + 1) * P, :],
                                  in_=out_t[:])


_NC_CACHE = None


def _get_nc():
    global _NC_CACHE
    if _NC_CACHE is None:
        _NC_CACHE = build_bass()
    return _NC_CACHE


def make_in_maps(inputs):
    in_maps = []
    for i in range(B):
        m = {
            "primals_1": np.ascontiguousarray(inputs["primals_1"], np.float32),
            "primals_2": np.ascontiguousarray(inputs["primals_2"], np.float32),
            "primals_3": np.ascontiguousarray(inputs["primals_3"], np.float32),
            "primals_4": np.ascontiguousarray(inputs["primals_4"], np.float32),
            "primals_5": np.ascontiguousarray(inputs["primals_5"], np.float32),
            "primals_6": np.ascontiguousarray(inputs["primals_6"], np.float32),
            "primals_7": np.ascontiguousarray(inputs["primals_7"], np.float32),
            "primals_8": np.ascontiguousarray(inputs["primals_8"], np.float32),
            "primals_9": np.ascontiguousarray(inputs["primals_9"][i], np.float32),
            "primals_10": np.ascontiguousarray(inputs["primals_10"][i], np.float32),
        }
        in_maps.append(m)
    return in_maps


def kernel(**inputs) -> np.ndarray:
    nc = _get_nc()
    in_maps = make_in_maps(inputs)
    res = run_bass_kernel_spmd(nc, in_maps, core_ids=list(range(B)))
    out = np.stack([res.results[i]["out"] for i in range(B)], axis=0)
    return out.astype(np.float32)


if __name__ == "__main__":
    # smoke: build only
    nc = _get_nc()
    print("built ok")
